# revision 1
# baseline (speedup 1.0000x reference)
"""Trainium2 Bass kernel for the Capsule routing module (nn_Capsule_2224793059594).

Full inputs in, full output out. Data-parallel over batch: 32 batches -> 8
cores x 4 batches.

v2 architecture (single u_hat layout, PE-heavy routing):
  - Natural-layout projection on PE: psum[i, o-chunk] = sum_k uT[k,(b,i)]^T
    kmat[k, o-chunk], plus a 3rd identity-weighted pass streaming the
    pos-emb table (u_hat += pe2). Evicted once to bf16 uh [i, (b, o)].
  - Routing iteration 1 folded to host (c1 = mask/128): b2T = utf^T @ w1tf
    + peB1 computed with f32 PE matmuls.
  - Softmax over n runs in the natural [i, n] layout (fused Exp+sum on ACT),
    producing cT [i, n] directly.
  - Contraction (1) outputs[n,d] = sum_i cT[i,n] uh[i,(n,d)] runs on the PE
    as 4 col-tiled block-diagonal matmuls (M=32, N=2048); the diagonal is
    pulled from PSUM with 32 partition-strided copies.
  - Contraction (2) b3T[i,n] = sum_d o2[n,d] uh[i,(n,d)] runs on DVE:
    o2 is DMA-flattened to a single-partition row [1, 8192] and broadcast
    across partitions (partition-step-0 AP), multiply at bf16 2x, reduce
    over d -> [i, n], already in softmax layout.
"""

import contextlib

import numpy as np
import ml_dtypes

import concourse.bass as bass
import concourse.bacc as bacc
import concourse.tile as tile
from concourse import mybir
from concourse.bass_utils import run_bass_kernel_spmd

B, S, IND, N, D = 32, 128, 256, 128, 64
NCORES = 8
NB = B // NCORES  # batches per core
EPS = 1e-7
BF16 = mybir.dt.bfloat16
F32 = mybir.dt.float32
AF = mybir.ActivationFunctionType
ALU = mybir.AluOpType
AX = mybir.AxisListType
bf = ml_dtypes.bfloat16


def _pe_table(s_, d_):
    pos = np.arange(s_, dtype=np.float32)[:, None]
    inv = (1.0 / np.power(np.float32(10000.0),
                          (2.0 * np.arange(d_ // 2, dtype=np.float32)) / np.float32(d_))
           ).astype(np.float32)
    ang = pos * inv[None, :]
    return np.stack([np.sin(ang), np.cos(ang)], axis=-1).reshape(s_, d_).astype(np.float32)


def _squash_np(s):
    ss = np.sum(s * s, axis=-1, keepdims=True)
    return (ss / (1.0 + ss) / np.sqrt(ss + EPS)) * s


def _build_device():
    nc = bacc.Bacc("TRN2", target_bir_lowering=False)

    kmat = nc.dram_tensor("kmat", [128, 2, N * D], BF16, kind="ExternalInput")
    pe2 = nc.dram_tensor("pe2", [128, N * D], BF16, kind="ExternalInput")
    idb = nc.dram_tensor("idb", [128, 128], BF16, kind="ExternalInput")
    ut = nc.dram_tensor("ut", [128, 2, NB, 128], BF16, kind="ExternalInput")
    utf = nc.dram_tensor("utf", [128, 2, NB, 128], F32, kind="ExternalInput")
    w1tf = nc.dram_tensor("w1tf", [128, 2, NB, 128], F32, kind="ExternalInput")
    peb1t = nc.dram_tensor("peb1t", [128, NB, 128], F32, kind="ExternalInput")
    mt = nc.dram_tensor("mt", [128, NB], F32, kind="ExternalInput")
    outd = nc.dram_tensor("out", [NB, 128, D], F32, kind="ExternalOutput")

    NCHUNK = N * D // 512  # 16 chunks of 512

    with tile.TileContext(nc, pool_alloc_mode="queue") as tc:
        with (
            tc.tile_pool(name="wrt", bufs=1) as wrt,
            tc.tile_pool(name="uhp", bufs=1) as uhp,
        ):
            ut_t = wrt.tile([128, 2, NB, 128], BF16)
            utf_t = wrt.tile([128, 2, NB, 128], F32)
            w1tf_t = wrt.tile([128, 2, NB, 128], F32)
            peb1_t = wrt.tile([128, NB, 128], F32)
            mt_t = wrt.tile([128, NB], F32)
            idb_t = wrt.tile([128, 128], BF16)
            ostage = wrt.tile([128, NB, D], F32)
            eps_t = wrt.tile([128, 1], F32)
            nc.vector.memset(eps_t[:], EPS)
            nc.sync.dma_start(out=ut_t[:], in_=ut[:])
            nc.sync.dma_start(out=utf_t[:], in_=utf[:])
            nc.sync.dma_start(out=w1tf_t[:], in_=w1tf[:])
            nc.sync.dma_start(out=peb1_t[:], in_=peb1t[:])
            nc.sync.dma_start(out=mt_t[:], in_=mt[:])
            nc.sync.dma_start(out=idb_t[:], in_=idb[:])

            uh = uhp.tile([128, NB, N * D], BF16)  # [i, b, (n d)]

            # routing pools open for the whole kernel; projection pools on
            # an ExitStack so their SBUF/PSUM frees up for phase 3
            proj_stack = contextlib.ExitStack()
            late_stack = contextlib.ExitStack()
            with (
                tc.tile_pool(name="rbig", bufs=1) as rbig,
                tc.tile_pool(name="rsm", bufs=3) as rsm,
                tc.tile_pool(name="rst", bufs=4) as rst,
                tc.tile_pool(name="pblk", bufs=1, space="PSUM") as pblk,
                tc.tile_pool(name="dscr", bufs=2, space="DRAM") as dscr,
            ):
                wproj = proj_stack.enter_context(tc.tile_pool(name="wproj", bufs=1))
                pproj = proj_stack.enter_context(
                    tc.tile_pool(name="pproj", bufs=1, space="PSUM"))
                km_t = wproj.tile([128, 2, N * D], BF16)
                pe_t = wproj.tile([128, N * D], BF16)
                # load in o-slabs so the chunk loop can start early
                for c0 in range(0, NCHUNK, 2):
                    sl = slice(c0 * 512, (c0 + 2) * 512)
                    nc.sync.dma_start(out=km_t[:, :, sl], in_=kmat[:, :, sl])
                    nc.sync.dma_start(out=pe_t[:, sl], in_=pe2[:, sl])

                def project(b):
                    for c0 in range(0, NCHUNK, 2):
                        sls = [slice(c * 512, (c + 1) * 512)
                               for c in (c0, c0 + 1)]
                        pss = [pproj.tile([128, 512], F32, tag="ps", bufs=3,
                                          name=f"ps_{b}_{c0}_{z}")
                               for z in range(2)]
                        for k in range(2):
                            for ps, sl in zip(pss, sls):
                                nc.tensor.matmul(ps[:], ut_t[:, k, b, :],
                                                 km_t[:, k, sl],
                                                 start=(k == 0), stop=False)
                        for ps, sl in zip(pss, sls):
                            nc.tensor.matmul(ps[:], idb_t[:], pe_t[:, sl],
                                             start=False, stop=True)
                        for z, (ps, sl) in enumerate(zip(pss, sls)):
                            if (c0 // 2 + z) % 4 == 3:
                                nc.vector.tensor_copy(uh[:, b, sl], ps[:])
                            else:
                                nc.scalar.copy(uh[:, b, sl], ps[:])

                # ---------------- routing ----------------
                uh4 = uh[:].rearrange("p b (n d) -> p b n d", d=D)

                def softmax_to_cT(bT_ap, b, tag="cT"):
                    """softmax over n (free) of bT [i, n] * mask -> cT [i, n] bf16."""
                    e = rsm.tile([128, 128], F32, tag="e")
                    den = rsm.tile([128, 1], F32, tag="den")
                    mx = rsm.tile([128, 1], F32, tag="mx")
                    nc.vector.tensor_reduce(mx[:], bT_ap, axis=AX.X, op=ALU.max)
                    nmx = rsm.tile([128, 1], F32, tag="nmx")
                    nc.vector.tensor_scalar_mul(nmx[:], mx[:], -1.0)
                    nc.scalar.activation(e[:], bT_ap, AF.Exp, bias=nmx[:],
                                         accum_out=den[:])
                    rden = rsm.tile([128, 1], F32, tag="rden")
                    nc.vector.reciprocal(rden[:], den[:])
                    rm = rsm.tile([128, 1], F32, tag="rm")
                    nc.vector.tensor_mul(rm[:], rden[:], mt_t[:, b:b + 1])
                    cT = rst.tile([128, 128], BF16, tag=tag)
                    nc.vector.tensor_scalar_mul(cT[:], e[:], rm[:])
                    return cT

                def contract1_pe(cT, b, pre_ap, pool, scr_eng):
                    """pre[n, d] = sum_i cT[i, n] * uh[i, b, (n, d)] via 4
                    col-tiled block-diagonal matmuls + diagonal extraction."""
                    ps = pool.tile([128, 32 * D], F32, tag="blk")
                    mms = [[None] * 4 for _ in range(4)]
                    for j in range(4):
                        nsl = slice(32 * j, 32 * (j + 1))
                        for q in range(4):  # psum bank-sized N=512 pieces
                            qn = slice(32 * j + 8 * q, 32 * j + 8 * (q + 1))
                            mms[j][q] = nc.tensor.matmul(
                                ps[nsl, 512 * q:512 * (q + 1)],
                                cT[:, nsl], uh4[:, b, qn, :],
                                start=True, stop=True,
                                tile_position=(0, 32 * j))
                    # diagonal extraction via DMA: dump PSUM to flat DRAM, then
                    # gather the diagonal (partition 32j+r's row lives at cols
                    # r*64..r*64+64 -> flat j*65536 + r*2112 + d)
                    scr = rbig.tile([128, 32 * D], F32, tag="scr", bufs=2)
                    if scr_eng == "act":
                        nc.scalar.copy(scr[:], ps[:])
                    else:
                        nc.vector.tensor_copy(scr[:], ps[:])
                    d1 = dscr.tile([128, 32 * D], F32, tag="d1")
                    nc.sync.dma_start(out=d1[:], in_=scr[:])
                    for j in range(4):
                        src = bass.AP(tensor=d1.tensor,
                                      offset=d1[:].offset + j * 32 * 32 * D,
                                      ap=[[32 * D + D, 32], [1, D]])
                        nc.sync.dma_start(out=pre_ap[32 * j:32 * (j + 1), :],
                                          in_=src)

                def squash_dev(pre, out_f32_ap=None, out_bf_ap=None):
                    sq = rsm.tile([128, D], F32, tag="sq")
                    ss = rsm.tile([128, 1], F32, tag="ss")
                    nc.scalar.activation(sq[:], pre[:], AF.Square, accum_out=ss[:])
                    srt = rsm.tile([128, 1], F32, tag="srt")
                    nc.scalar.activation(srt[:], ss[:], AF.Sqrt, bias=eps_t[:])
                    ssp = rsm.tile([128, 1], F32, tag="ssp")
                    nc.vector.tensor_scalar_add(ssp[:], ss[:], 1.0)
                    dn = rsm.tile([128, 1], F32, tag="dn")
                    nc.vector.tensor_mul(dn[:], srt[:], ssp[:])
                    rcp = rsm.tile([128, 1], F32, tag="rcp")
                    nc.vector.reciprocal(rcp[:], dn[:])
                    scl = rsm.tile([128, 1], F32, tag="scl")
                    nc.vector.tensor_mul(scl[:], ss[:], rcp[:])
                    if out_f32_ap is not None:
                        nc.vector.tensor_scalar_mul(out_f32_ap, pre[:], scl[:])
                    if out_bf_ap is not None:
                        nc.vector.tensor_scalar_mul(out_bf_ap, pre[:], scl[:])

                # stage-major emission: run each stage across all batches so
                # engine instruction streams interleave batches
                cT2s, pre2s, o2bs, b3Ts, cT3s, pre3s = ({} for _ in range(6))

                for b in range(NB):
                    # projection of this batch, then iter1 agreement via host
                    # w1 (f32 matmul), softmax, and the iter-2 (1)-contraction
                    project(b)
                    bps = pproj.tile([128, 128], F32, tag="bps")
                    nc.tensor.matmul(bps[:], utf_t[:, 0, b, :], w1tf_t[:, 0, b, :],
                                     start=True, stop=False)
                    nc.tensor.matmul(bps[:], utf_t[:, 1, b, :], w1tf_t[:, 1, b, :],
                                     start=False, stop=True)
                    b2T = rsm.tile([128, 128], F32, tag="b2T")
                    nc.vector.tensor_add(b2T[:], bps[:], peb1_t[:, b, :])
                    cT2s[b] = softmax_to_cT(b2T[:], b, tag="cT2")
                    pre2s[b] = rst.tile([128, D], F32, tag="pre2", name=f"pre2_{b}")
                    contract1_pe(cT2s[b], b, pre2s[b][:], pblk,
                                 "act" if b % 2 == 0 else "dve")

                proj_stack.close()
                pblk2 = late_stack.enter_context(
                    tc.tile_pool(name="pblk2", bufs=1, space="PSUM"))

                for b in range(NB):
                    o2bs[b] = rst.tile([128, D], BF16, tag="ob", name=f"ob_{b}")
                    squash_dev(pre2s[b], out_bf_ap=o2bs[b][:])
                    # flatten o2b [n, d] -> DRAM row, then broadcast-load to
                    # all 128 partitions (step-0 over flat DRAM is allowed)
                    o2d = dscr.tile([N * D], BF16, tag="o2d")
                    nc.sync.dma_start(out=o2d[:], in_=o2bs[b][:])
                    o2bc = rbig.tile([128, N * D], BF16, tag="o2bc", bufs=2)
                    rep = bass.AP(tensor=o2d.tensor, offset=o2d[:].offset,
                                  ap=[[0, 128]] + [list(x) for x in o2d[:].ap])
                    nc.sync.dma_start(out=o2bc[:], in_=rep)
                    tmp2 = rbig.tile([128, N * D], BF16, tag="tmp2")
                    nc.vector.tensor_mul(tmp2[:], uh[:, b, :], o2bc[:])
                    b3Ts[b] = rst.tile([128, 128], F32, tag="b3T", name=f"b3T_{b}")
                    nc.vector.tensor_reduce(
                        b3Ts[b][:], tmp2[:].rearrange("p (n d) -> p n d", d=D),
                        axis=AX.X, op=ALU.add)
                    cT3s[b] = softmax_to_cT(b3Ts[b][:], b, tag="cT3")

                for b in range(NB):
                    pre3s[b] = rst.tile([128, D], F32, tag="pre3", name=f"pre3_{b}")
                    contract1_pe(cT3s[b], b, pre3s[b][:],
                                 pblk if b % 2 == 0 else pblk2,
                                 "act" if b % 2 == 1 else "dve")

                for b in range(NB):
                    squash_dev(pre3s[b], out_f32_ap=ostage[:, b, :])
                    nc.sync.dma_start(out=outd[b], in_=ostage[:, b, :])
                late_stack.close()

    nc.finalize()
    return nc


_NC_CACHE = None


def _host_prep(u_vecs, mask, W):
    pe1 = _pe_table(N, D)                        # [n, d]
    pe2 = _pe_table(S, N * D).reshape(S, N, D)   # [i, n, d]
    kmat = (W[0][:, None, :] + pe1[None, :, :]).astype(np.float32)  # [256, n, d]

    # iteration-1 shortcut (c1 = mask/128):
    mu = np.einsum('bi,biI->bI', mask, u_vecs)
    s1 = (np.einsum('bI,Ind->bnd', mu, kmat)
          + np.einsum('bi,ind->bnd', mask, pe2)) / np.float32(N)
    o1 = _squash_np(s1.astype(np.float32))
    w1 = np.einsum('Ind,bnd->bnI', kmat, o1)
    peb1 = np.einsum('ind,bnd->ibn', pe2, o1)

    kmat_h = np.ascontiguousarray(
        kmat.reshape(2, 128, N * D).transpose(1, 0, 2)).astype(bf)  # [p, k, o]
    pe2_h = np.ascontiguousarray(pe2.reshape(S, N * D)).astype(bf)  # [i, o]
    idb_h = np.eye(128, dtype=np.float32).astype(bf)

    shared = dict(kmat=kmat_h, pe2=pe2_h, idb=idb_h)

    in_maps = []
    for c in range(NCORES):
        sl = slice(c * NB, (c + 1) * NB)
        u_c = u_vecs[sl]
        utf_h = np.ascontiguousarray(
            u_c.transpose(2, 0, 1).reshape(2, 128, NB, 128)
               .transpose(1, 0, 2, 3)).astype(np.float32)  # [p, k, b, i]
        ut_h = utf_h.astype(bf)
        w1_c = w1[sl]
        w1tf_h = np.ascontiguousarray(
            w1_c.transpose(2, 0, 1).reshape(2, 128, NB, 128)
                .transpose(1, 0, 2, 3)).astype(np.float32)  # [p, k, b, n]
        peb1_h = np.ascontiguousarray(peb1[:, sl, :]).astype(np.float32)
        mt_h = np.ascontiguousarray(mask[sl].T).astype(np.float32)
        m = dict(shared)
        m.update(ut=ut_h, utf=utf_h, w1tf=w1tf_h, peb1t=peb1_h, mt=mt_h)
        in_maps.append(m)
    return in_maps


def kernel(u_vecs, mask, W):
    global _NC_CACHE
    u_vecs = np.asarray(u_vecs, dtype=np.float32)
    mask = np.asarray(mask, dtype=np.float32)
    W = np.asarray(W, dtype=np.float32)

    in_maps = _host_prep(u_vecs, mask, W)
    if _NC_CACHE is None:
        _NC_CACHE = _build_device()
    res = run_bass_kernel_spmd(_NC_CACHE, in_maps, core_ids=list(range(NCORES)))
    outs = [np.asarray(r["out"], dtype=np.float32) for r in res.results]
    return np.concatenate(outs, axis=0)



# revision 15
# speedup vs baseline: 1.1273x; 1.1273x over previous
"""Trainium2 Bass kernel for the Capsule routing module (nn_Capsule_2224793059594).

Full inputs in, full output out. Data-parallel over batch: 32 batches -> 8
cores x 4 batches.

v3 architecture (dense PE conveyor, cheap diagonal extraction, tree reduce):
  - Projection per batch in 8 PSUM groups of 1024 cols, k-major inside a
    group (ut_k0, ut_k1, identity@pe2 passes back-to-back) so the PE streams
    densely and HAM stays warm. Groups evicted to bf16 uh by ACT (cast copy).
  - Routing iteration 1 folded to host (c1 = mask/128): b2T = utf^T @ w1tf
    + peB1 with f32 PE matmuls.
  - Softmax over n in natural [i, n] layout (fused Exp+sum on ACT) -> cT.
  - Contraction (1) outputs[n,d] = sum_i cT[i,n] uh[i,(n,d)] on the PE as 16
    M=8 block-diagonal matmuls into ONE psum bank [128, 512] (row 8j+s keeps
    cols 64s..64s+64). Diagonal extracted via bf16 dump [128,512] to DRAM +
    one 3D-AP gather DMA (flat offset = 4096j + 576s + d).
  - squash uses only the natural_log_exp ACT table set (sqrt = exp(0.5 ln)).
  - Contraction (2) b3T[i,n] = sum_d o2[n,d] uh[i,(n,d)] on DVE: o2 is
    broadcast via DRAM round trip in 2 halves, then bf16 2x multiply and a
    6-level pairwise-add tree (all 2x) instead of a 1x tensor_reduce.
  - Stage-major emission across batches keeps every engine's FIFO free of
    cross-batch priority inversions.
"""

import numpy as np
import ml_dtypes

import concourse.bass as bass
import concourse.bacc as bacc
import concourse.tile as tile
from concourse import mybir
from concourse.bass_utils import run_bass_kernel_spmd

B, S, IND, N, D = 32, 128, 256, 128, 64
NCORES = 8
NB = B // NCORES  # batches per core
EPS = 1e-7
BF16 = mybir.dt.bfloat16
F32 = mybir.dt.float32
AF = mybir.ActivationFunctionType
ALU = mybir.AluOpType
AX = mybir.AxisListType
bf = ml_dtypes.bfloat16

NGRP = 8           # projection psum groups per batch
GW = N * D // NGRP  # 1024 cols per group
C1M = 8            # contract1 weight block width (M)


def _pe_table(s_, d_):
    pos = np.arange(s_, dtype=np.float32)[:, None]
    inv = (1.0 / np.power(np.float32(10000.0),
                          (2.0 * np.arange(d_ // 2, dtype=np.float32)) / np.float32(d_))
           ).astype(np.float32)
    ang = pos * inv[None, :]
    return np.stack([np.sin(ang), np.cos(ang)], axis=-1).reshape(s_, d_).astype(np.float32)


def _squash_np(s):
    ss = np.sum(s * s, axis=-1, keepdims=True)
    return (ss / (1.0 + ss) / np.sqrt(ss + EPS)) * s


def _build_device():
    nc = bacc.Bacc("TRN2", target_bir_lowering=False)

    kmat = nc.dram_tensor("kmat", [128, 2, N * D], BF16, kind="ExternalInput")
    pe2 = nc.dram_tensor("pe2", [128, N * D], BF16, kind="ExternalInput")
    idb = nc.dram_tensor("idb", [128, 128], BF16, kind="ExternalInput")
    ut = nc.dram_tensor("ut", [128, 2, NB, 128], BF16, kind="ExternalInput")
    b2t = nc.dram_tensor("b2t", [128, NB, 128], F32, kind="ExternalInput")
    mt = nc.dram_tensor("mt", [128, NB], F32, kind="ExternalInput")
    outd = nc.dram_tensor("out", [NB, 128, D], F32, kind="ExternalOutput")

    import contextlib

    with tile.TileContext(nc, pool_alloc_mode="queue") as tc:
        proj_stack = contextlib.ExitStack()
        late_stack = contextlib.ExitStack()
        with (
            tc.tile_pool(name="wrt", bufs=1) as wrt,
            tc.tile_pool(name="uhp", bufs=1) as uhp,
            tc.tile_pool(name="rsm", bufs=3) as rsm,
            tc.tile_pool(name="rst", bufs=4) as rst,
            tc.tile_pool(name="tre", bufs=2) as tre,
            tc.tile_pool(name="obc", bufs=2) as obc,
            tc.tile_pool(name="pc1", bufs=1, space="PSUM") as pc1,
            tc.tile_pool(name="dscr", bufs=3, space="DRAM") as dscr,
        ):
            pproj = proj_stack.enter_context(
                tc.tile_pool(name="pproj", bufs=2, space="PSUM"))
            wproj = proj_stack.enter_context(tc.tile_pool(name="wproj", bufs=1))

            ut_t = wrt.tile([128, 2, NB, 128], BF16)
            b2t_t = wrt.tile([128, NB, 128], F32)
            mt_t = wrt.tile([128, NB], F32)
            idb_t = wrt.tile([128, 128], BF16)
            ostage = wrt.tile([128, NB, D], F32)
            eps_t = wrt.tile([128, 1], F32)
            nc.vector.memset(eps_t[:], EPS)
            nc.sync.dma_start(out=ut_t[:], in_=ut[:])
            nc.sync.dma_start(out=b2t_t[:], in_=b2t[:])
            nc.sync.dma_start(out=mt_t[:], in_=mt[:])
            nc.sync.dma_start(out=idb_t[:], in_=idb[:])

            km_t = wproj.tile([128, 2, N * D], BF16)
            pe_t = wproj.tile([128, N * D], BF16)
            # load kmat/pe2 in o-slabs so batch 0's projection starts early
            for c0 in range(0, N * D, 2048):
                sl = slice(c0, c0 + 2048)
                nc.sync.dma_start(out=km_t[:, :, sl], in_=kmat[:, :, sl])
                nc.sync.dma_start(out=pe_t[:, sl], in_=pe2[:, sl])

            uh = uhp.tile([128, NB, N * D], BF16)  # [i, b, (n d)]
            uh4 = uh[:].rearrange("p b (n d) -> p b n d", d=D)

            # ---------- stage helpers ----------

            def project(b):
                for g in range(NGRP):
                    ps = pproj.tile([128, GW], F32, tag="ps", name=f"ps_{b}_{g}")
                    for k in range(2):
                        for q in range(2):
                            sl = slice(g * GW + q * 512, g * GW + (q + 1) * 512)
                            nc.tensor.matmul(ps[:, q * 512:(q + 1) * 512],
                                             ut_t[:, k, b, :], km_t[:, k, sl],
                                             start=(k == 0), stop=False)
                    for q in range(2):
                        sl = slice(g * GW + q * 512, g * GW + (q + 1) * 512)
                        nc.tensor.matmul(ps[:, q * 512:(q + 1) * 512],
                                         idb_t[:], pe_t[:, sl],
                                         start=False, stop=True)
                    nc.scalar.copy(uh[:, b, g * GW:(g + 1) * GW], ps[:])

            def softmax_to_cT(bT_ap, b, tag):
                """softmax over n (free) of bT [i, n] * mask -> cT [i, n] bf16."""
                e = rsm.tile([128, 128], F32, tag="e")
                den = rsm.tile([128, 1], F32, tag="den")
                mx = rsm.tile([128, 1], F32, tag="mx")
                nc.vector.tensor_reduce(mx[:], bT_ap, axis=AX.X, op=ALU.max)
                nmx = rsm.tile([128, 1], F32, tag="nmx")
                nc.vector.tensor_scalar_mul(nmx[:], mx[:], -1.0)
                nc.scalar.activation(e[:], bT_ap, AF.Exp, bias=nmx[:],
                                     accum_out=den[:])
                rden = rsm.tile([128, 1], F32, tag="rden")
                nc.vector.reciprocal(rden[:], den[:])
                rm = rsm.tile([128, 1], F32, tag="rm")
                nc.vector.tensor_mul(rm[:], rden[:], mt_t[:, b:b + 1])
                cT = rst.tile([128, 128], BF16, tag=tag)
                nc.vector.tensor_scalar_mul(cT[:], e[:], rm[:])
                return cT

            def contract1(cT, b, pre_ap, it, pool, scr_eng="act"):
                """pre[n, d] = sum_i cT[i, n] uh[i, b, (n d)] via 16 col-tiled
                M=32 block matmuls + bf16 diag dump/gather (useful element of
                psum row r within block j sits at col 64 r + d)."""
                ps = pool.tile([128, 32 * D], F32, tag="c1ps",
                               name=f"c1_{b}_{it}")
                for q in range(4):
                    for j in range(4):  # col-group interleave
                        nsl = slice(32 * j, 32 * (j + 1))
                        qn = slice(32 * j + 8 * q, 32 * j + 8 * (q + 1))
                        nc.tensor.matmul(ps[nsl, 512 * q:512 * (q + 1)],
                                         cT[:, nsl], uh4[:, b, qn, :],
                                         start=True, stop=True,
                                         tile_position=(0, 32 * j))
                scr = rst.tile([128, 32 * D], BF16, tag="scr", bufs=2,
                               name=f"scr_{b}_{it}")
                if scr_eng == "act":
                    nc.scalar.copy(scr[:], ps[:])
                else:
                    nc.vector.tensor_copy(scr[:], ps[:])
                d1 = dscr.tile([128, 32 * D], BF16, tag="d1")
                nc.sync.dma_start(out=d1[:], in_=scr[:])
                # flat elem offset of diag: 65536 j + 2112 r + d
                src = bass.AP(tensor=d1.tensor, offset=d1[:].offset,
                              ap=[[32 * 2048, 4], [2048 + D, 32], [1, D]])
                nc.sync.dma_start(out=pre_ap, in_=src)

            def squash(pre, out_f32_ap=None, out_bf_ap=None):
                sq = rsm.tile([128, D], F32, tag="sq")
                ss = rsm.tile([128, 1], F32, tag="ss")
                nc.scalar.activation(sq[:], pre[:], AF.Square, accum_out=ss[:])
                # sqrt(ss+eps) = exp(0.5 * ln(ss+eps)) -- stays in the
                # natural_log_exp table set (no table switch vs Sqrt)
                lg = rsm.tile([128, 1], F32, tag="lg")
                nc.scalar.activation(lg[:], ss[:], AF.Ln, bias=eps_t[:])
                srt = rsm.tile([128, 1], F32, tag="srt")
                nc.scalar.activation(srt[:], lg[:], AF.Exp, scale=0.5)
                ssp = rsm.tile([128, 1], F32, tag="ssp")
                nc.vector.tensor_scalar_add(ssp[:], ss[:], 1.0)
                dn = rsm.tile([128, 1], F32, tag="dn")
                nc.vector.tensor_mul(dn[:], srt[:], ssp[:])
                rcp = rsm.tile([128, 1], F32, tag="rcp")
                nc.vector.reciprocal(rcp[:], dn[:])
                scl = rsm.tile([128, 1], F32, tag="scl")
                nc.vector.tensor_mul(scl[:], ss[:], rcp[:])
                if out_f32_ap is not None:
                    nc.vector.tensor_scalar_mul(out_f32_ap, pre[:], scl[:])
                if out_bf_ap is not None:
                    nc.vector.tensor_scalar_mul(out_bf_ap, pre[:], scl[:])

            def b3_half(b, o2d, h, b3T):
                """b3T[:, 64h:64h+64] = sum_d uh[i, n, d] o2[n, d] for the
                n-half h: bf16 2x multiply + pairwise-add tree."""
                cols = slice(h * 4096, (h + 1) * 4096)
                o2bc = obc.tile([128, 4096], BF16, tag="o2bc",
                                name=f"o2bc_{b}_{h}")
                rep = bass.AP(tensor=o2d.tensor,
                              offset=o2d[:].offset + h * 4096,
                              ap=[[0, 128], [1, 4096]])
                nc.sync.dma_start(out=o2bc[:], in_=rep)
                t0 = tre.tile([128, 4096], BF16, tag="t0", name=f"t0_{b}_{h}")
                nc.vector.tensor_mul(t0[:], uh[:, b, cols], o2bc[:])
                cur, w = t0, 32
                while w >= 2:
                    nxt = tre.tile([128, 64 * w], BF16, tag=f"t{w}",
                                   name=f"t{w}_{b}_{h}")
                    a3 = cur[:].rearrange("p (n d) -> p n d", d=2 * w)
                    nc.vector.tensor_add(nxt[:].rearrange("p (n d) -> p n d", d=w),
                                         a3[:, :, 0:w], a3[:, :, w:2 * w])
                    cur, w = nxt, w // 2
                a3 = cur[:].rearrange("p (n d) -> p n d", d=2)
                out3 = b3T[:, 64 * h:64 * (h + 1)].rearrange(
                    "p (n o) -> p n o", o=1)
                nc.vector.tensor_add(out3, a3[:, :, 0:1], a3[:, :, 1:2])

            # ---------- wave A: projection + iter-2 contraction ----------
            cT2s, pre2s, o2bs, b3Ts, cT3s, pre3s = ({} for _ in range(6))
            for b in range(NB):
                project(b)
                cT2s[b] = softmax_to_cT(b2t_t[:, b, :], b, tag="cT2")
                pre2s[b] = rst.tile([128, D], BF16, tag="pre2", name=f"pre2_{b}")
                contract1(cT2s[b], b, pre2s[b][:], 2, pc1,
                          "act" if b % 2 else "dve")

            # ---------- wave B: squash2, o2 broadcast, b3T, softmax3 ----------
            o2ds = {}
            for b in range(NB):
                o2bs[b] = rst.tile([128, D], BF16, tag="ob", name=f"ob_{b}")
                squash(pre2s[b], out_bf_ap=o2bs[b][:])
                o2ds[b] = dscr.tile([N * D], BF16, tag=f"o2d{b}",
                                    name=f"o2d_{b}")
                nc.sync.dma_start(out=o2ds[b][:], in_=o2bs[b][:])
            proj_stack.close()
            pc2 = late_stack.enter_context(
                tc.tile_pool(name="pc2", bufs=1, space="PSUM"))
            for b in range(NB):
                b3Ts[b] = rst.tile([128, 128], F32, tag="b3T", name=f"b3T_{b}")
                for h in range(2):
                    b3_half(b, o2ds[b], h, b3Ts[b][:])
                cT3s[b] = softmax_to_cT(b3Ts[b][:], b, tag="cT3")

            # ---------- wave C: iter-3 contraction + squash + out ----------
            for b in range(NB):
                pre3s[b] = rst.tile([128, D], BF16, tag="pre3", name=f"pre3_{b}")
                contract1(cT3s[b], b, pre3s[b][:], 3,
                          pc2 if b % 2 == 0 else pc1,
                          "act" if b % 2 == 0 else "dve")
            for b in range(NB):
                squash(pre3s[b], out_f32_ap=ostage[:, b, :])
                nc.sync.dma_start(out=outd[b], in_=ostage[:, b, :])
            late_stack.close()

    nc.finalize()
    return nc


_NC_CACHE = None


def _host_prep(u_vecs, mask, W):
    pe1 = _pe_table(N, D)                        # [n, d]
    pe2 = _pe_table(S, N * D).reshape(S, N, D)   # [i, n, d]
    kmat = (W[0][:, None, :] + pe1[None, :, :]).astype(np.float32)  # [256, n, d]

    # iteration-1 shortcut (c1 = mask/128): fold the whole first routing
    # iteration (uniform softmax) plus the iter-2 agreement logits to host.
    mu = np.einsum('bi,biI->bI', mask, u_vecs)
    s1 = (np.einsum('bI,Ind->bnd', mu, kmat)
          + np.einsum('bi,ind->bnd', mask, pe2)) / np.float32(N)
    o1 = _squash_np(s1.astype(np.float32))
    w1 = np.einsum('Ind,bnd->bnI', kmat, o1)
    peb1 = np.einsum('ind,bnd->ibn', pe2, o1)
    # b2[b, i, n] = sum_I u[b, i, I] w1[b, n, I] + peb1[i, b, n]
    b2 = (np.einsum('biI,bnI->ibn', u_vecs, w1) + peb1).astype(np.float32)

    kmat_h = np.ascontiguousarray(
        kmat.reshape(2, 128, N * D).transpose(1, 0, 2)).astype(bf)  # [p, k, o]
    pe2_h = np.ascontiguousarray(pe2.reshape(S, N * D)).astype(bf)  # [i, o]
    idb_h = np.eye(128, dtype=np.float32).astype(bf)

    shared = dict(kmat=kmat_h, pe2=pe2_h, idb=idb_h)

    in_maps = []
    for c in range(NCORES):
        sl = slice(c * NB, (c + 1) * NB)
        u_c = u_vecs[sl]
        ut_h = np.ascontiguousarray(
            u_c.transpose(2, 0, 1).reshape(2, 128, NB, 128)
               .transpose(1, 0, 2, 3)).astype(bf)  # [p, k, b, i]
        b2_h = np.ascontiguousarray(b2[:, sl, :]).astype(np.float32)
        mt_h = np.ascontiguousarray(mask[sl].T).astype(np.float32)
        m = dict(shared)
        m.update(ut=ut_h, b2t=b2_h, mt=mt_h)
        in_maps.append(m)
    return in_maps


def kernel(u_vecs, mask, W):
    global _NC_CACHE
    u_vecs = np.asarray(u_vecs, dtype=np.float32)
    mask = np.asarray(mask, dtype=np.float32)
    W = np.asarray(W, dtype=np.float32)

    in_maps = _host_prep(u_vecs, mask, W)
    if _NC_CACHE is None:
        _NC_CACHE = _build_device()
    res = run_bass_kernel_spmd(_NC_CACHE, in_maps, core_ids=list(range(NCORES)))
    outs = [np.asarray(r["out"], dtype=np.float32) for r in res.results]
    return np.concatenate(outs, axis=0)
